# revision 8
# baseline (speedup 1.0000x reference)
import os
import sys

sys.path.insert(0, "/opt/trn_rl_repo")

import numpy as np
import ml_dtypes

N_NODES = 10000
NP = 10240          # padded node count (80 * 128)
F = 128
FO = 40
BN_EPS = 1e-5
NC_ = 8
RPC = NP // NC_     # 1280 rows per core
MB = RPC // 128     # 10 m-blocks per core
KB = NP // 128      # 80 k-blocks
R_RES = 40          # A k-blocks resident in SBUF across layers
CHUNKS = [(0, 512), (512, 512), (1024, 256)]

LAST_RESULTS = None
_cache = {}


def _build():
    import concourse.bass as bass
    import concourse.mybir as mybir
    from concourse.tile import TileContext
    from concourse.vector_clock import ScopedClock
    import bass_rust

    f32 = mybir.dt.float32
    bf16 = mybir.dt.bfloat16
    Alu = mybir.AluOpType
    ActF = mybir.ActivationFunctionType

    class TC(TileContext):
        # The stock final drain puts one sync-wait per outstanding semaphore
        # lane on a single instruction; >4 waits fails walrus codegen. Spill
        # the overflow onto chained nops ahead of the drain.
        def _drain_and_barrier(self, tick_clock, wait_clock):
            first = self.nc.sync.nop(nofuse=True, hint="final_wait")
            wait_clock.add_sem_waits(
                first.ins, ScopedClock({None: tick_clock.global_clock})
            )
            si = first.ins.sync_info
            if si is not None and len(si.on_wait) > 4:
                waits = list(si.on_wait)
                first.ins.sync_info = bass_rust.SyncInfo(
                    on_wait=waits[:4], on_update=list(si.on_update)
                )
                for i in range(4, len(waits), 4):
                    extra = self.nc.sync.nop(nofuse=True, hint=f"final_wait_{i}")
                    extra.ins.sync_info = bass_rust.SyncInfo(
                        on_wait=waits[i : i + 4], on_update=[]
                    )
            self.nc.sync.drain()
            self.nc.all_engine_barrier()
            assert self.sems is not None
            popped = self.nc._tile_sem_poison_stack.pop()
            assert popped is self._sem_poison
            self.nc.clear_and_free_semaphores(list(self.sems.allocated().values()))
            self.nc.all_engine_barrier()

    nc = bass.Bass(num_devices=NC_)
    a_in = nc.declare_dram_parameter("a", [KB, 128, RPC], bf16, isOutput=False)
    xz_in = nc.declare_dram_parameter("xz", [NC_, 128, MB, F], bf16, isOutput=False)
    w1_in = nc.declare_dram_parameter("w1", [F, F], f32, isOutput=False)
    w2_in = nc.declare_dram_parameter("w2", [F, F], f32, isOutput=False)
    w3_in = nc.declare_dram_parameter("w3", [F, FO], f32, isOutput=False)
    gb_in = nc.declare_dram_parameter("gb", [F, 4], f32, isOutput=False)
    bs_in = nc.declare_dram_parameter("bs", [3, F], f32, isOutput=False)
    rsm_in = nc.declare_dram_parameter("rsm", [2, RPC], f32, isOutput=False)
    mc_in = nc.declare_dram_parameter("mc", [128, MB], bf16, isOutput=False)
    out_p = nc.declare_dram_parameter("out", [RPC, FO], f32, isOutput=True)

    rg = [list(range(NC_))]
    zb = [nc.dram_tensor(f"zb{l}", [128, MB, F], bf16, kind="Internal") for l in range(2)]
    zg = [
        nc.dram_tensor(f"zg{l}", [NC_, 128, MB, F], bf16, kind="Internal", addr_space="Shared")
        for l in range(2)
    ]
    sti = [nc.dram_tensor(f"sti{l}", [F, 2], f32, kind="Internal") for l in range(2)]
    sto = [
        nc.dram_tensor(f"sto{l}", [F, 2], f32, kind="Internal", addr_space="Shared")
        for l in range(2)
    ]

    with TC(nc) as tc:
        with (
            tc.tile_pool(name="ares", bufs=R_RES) as pares,
            tc.tile_pool(name="astr", bufs=6) as pastr,
            tc.tile_pool(name="pz", bufs=18) as pz,
            tc.tile_pool(name="pst", bufs=2) as pst,
            tc.tile_pool(name="psq", bufs=2) as psq,
            tc.tile_pool(name="pcst", bufs=12) as pcst,
            tc.tile_pool(name="prs", bufs=1) as prs,
            tc.tile_pool(name="prhs", bufs=3) as prhs,
            tc.tile_pool(name="pmc", bufs=1) as pmc,
            tc.tile_pool(name="pout", bufs=3) as pout,
            tc.tile_pool(name="psS", bufs=3, space="PSUM") as psS,
            tc.tile_pool(name="psZ", bufs=2, space="PSUM") as psZ,
            tc.tile_pool(name="psT", bufs=2, space="PSUM") as psT,
        ):
            # ---- constants
            w1t = pcst.tile([F, F], f32, bufs=1)
            nc.sync.dma_start(w1t[:], w1_in[:])
            w2t = pcst.tile([F, F], f32, bufs=1)
            nc.sync.dma_start(w2t[:], w2_in[:])
            w3t = pcst.tile([F, F], f32, bufs=1)
            nc.sync.dma_start(w3t[:, 0:FO], w3_in[:])
            gbt = pcst.tile([F, 4], f32, bufs=1)
            nc.sync.dma_start(gbt[:], gb_in[:])
            rsmt = prs.tile([2, RPC], f32, bufs=1)
            nc.sync.dma_start(rsmt[:], rsm_in[:])
            mct = pmc.tile([128, MB], bf16, bufs=1)
            nc.sync.dma_start(mct[:], mc_in[:])
            rhs2 = []
            for l in range(3):
                t = prhs.tile([2, F], f32, name="rhs2t", bufs=3)
                if l == 0:
                    nc.vector.memset(t[0:1, :], 0.0)
                nc.sync.dma_start(t[1:2, :], bs_in[l : l + 1, :])
                rhs2.append(t)

            a_res = [None] * R_RES
            st_sb = [None, None]   # [F, 2+2] stats sbuf tiles per bn layer
            wp = [w1t, None, None]  # effective weight tile per layer
            bn_sc = [None, None]    # (scale, shift) APs per bn layer

            def spmm(l, ztiles):
                """S.T = (A_core @ Z).T accumulated in 3 PSUM chunks."""
                sts = [psS.tile([F, c], f32, name=f"stS{ci}", bufs=1) for ci, (_, c) in enumerate(CHUNKS)]
                for k in range(KB):
                    if k < R_RES:
                        if l == 0:
                            at = pares.tile([128, RPC], bf16, name="ares_t", bufs=R_RES)
                            nc.sync.dma_start(at[:], a_in[k])
                            a_res[k] = at
                        at = a_res[k]
                    else:
                        at = pastr.tile([128, RPC], bf16, name="astr_t", bufs=6)
                        nc.sync.dma_start(at[:], a_in[k])
                    lhs = ztiles[k // MB][:, k % MB, :]
                    for ci, (off, cw) in enumerate(CHUNKS):
                        nc.tensor.matmul(
                            sts[ci][:],
                            lhs,
                            at[:, off : off + cw],
                            start=(k == 0),
                            stop=(k == KB - 1),
                        )
                return sts

            def bn_fold(l):
                """After AllReduce of layer-l stats: compute scale/shift, fold
                scale into next-layer weights. Emits DVE/ACT ops only."""
                st = st_sb[l]
                bn = pcst.tile([F, 8], f32, name="bn", bufs=2)
                mean, e2, msq, var = bn[:, 0:1], bn[:, 1:2], bn[:, 2:3], bn[:, 3:4]
                sd, rs, scale, shift = bn[:, 4:5], bn[:, 5:6], bn[:, 6:7], bn[:, 7:8]
                g_col = gbt[:, 2 * l : 2 * l + 1]
                b_col = gbt[:, 2 * l + 1 : 2 * l + 2]
                nc.vector.tensor_scalar_mul(mean, st[:, 2:3], 1.0 / N_NODES)
                nc.vector.tensor_scalar_mul(e2, st[:, 3:4], 1.0 / N_NODES)
                nc.vector.tensor_mul(msq, mean, mean)
                nc.vector.tensor_sub(var, e2, msq)
                nc.vector.tensor_scalar_add(var, var, BN_EPS)
                nc.scalar.sqrt(sd, var)
                nc.vector.reciprocal(rs, sd)
                nc.vector.tensor_mul(scale, g_col, rs)
                # shift = beta - mean*scale
                nc.vector.scalar_tensor_tensor(
                    shift, mean, scale, b_col, Alu.mult, Alu.subtract
                )
                nc.vector.tensor_scalar_mul(shift, shift, -1.0)
                bn_sc[l] = (scale, shift)
                # fold scale into next layer's W
                wsrc = w2t if l == 0 else w3t
                cols = F if l == 0 else FO
                w_eff = pcst.tile([F, F], f32, name="w_eff", bufs=2)
                nc.vector.tensor_scalar_mul(w_eff[:, 0:cols], wsrc[:, 0:cols], scale)
                wp[l + 1] = w_eff

            def v_matmul(l):
                """v = shift.T @ W_next  -> row 0 of rhs2[l+1]. PE op; emit
                after the next layer's SpMM matmuls so PE never stalls on it."""
                _, shift = bn_sc[l]
                wsrc = w2t if l == 0 else w3t
                cols = F if l == 0 else FO
                vp = psZ.tile([1, F], f32, name="vp", bufs=1)
                nc.tensor.matmul(vp[:, 0:cols], shift, wsrc[:, 0:cols], start=True, stop=True)
                nc.scalar.copy(rhs2[l + 1][0:1, 0:cols], vp[:, 0:cols])

            def linear_block(l, stt, zown):
                """zpre = S @ W_eff + rowsum x v + b; relu (l<2) into zown or
                copy to output tiles (l==2)."""
                cols = F if l < 2 else FO
                for m in range(MB):
                    zp = psZ.tile([128, cols], f32, name="zp", bufs=2)
                    nc.tensor.matmul(
                        zp[:],
                        stt[:, m * 128 : (m + 1) * 128],
                        wp[l][:, 0:cols],
                        start=True,
                        stop=False,
                    )
                    nc.tensor.matmul(
                        zp[:],
                        rsmt[:, m * 128 : (m + 1) * 128],
                        rhs2[l][:, 0:cols],
                        start=False,
                        stop=True,
                    )
                    if l < 2:
                        nc.scalar.activation(zown[:, m, :], zp[:], ActF.Relu)
                    else:
                        ot = pout.tile([128, FO], f32, name="ot", bufs=3)
                        nc.scalar.copy(ot[:], zp[:])
                        nc.sync.dma_start(out_p[m * 128 : (m + 1) * 128, :], ot[:])

            def stats_and_comms(l, zown):
                """AllGather z, then BN stats + AllReduce. Returns next Z tiles."""
                nc.sync.dma_start(zb[l][:], zown[:])
                nc.gpsimd.collective_compute(
                    "AllGather", Alu.bypass, replica_groups=rg,
                    ins=[zb[l][:]], outs=[zg[l][:]],
                )
                zt_next = []
                for r in range(NC_):
                    t = pz.tile([128, MB, F], bf16, name="zt", bufs=16)
                    nc.sync.dma_start(t[:], zg[l][r])
                    zt_next.append(t)
                # stats: column sums of z and z^2 over this core's real rows
                sum_ps = psT.tile([F, 1], f32, name="sum_ps", bufs=1)
                sq_ps = psT.tile([F, 1], f32, name="sq_ps", bufs=1)
                for m in range(MB):
                    zsl = zown[:, m, :]
                    zsq = psq.tile([128, F], bf16, name="zsq", bufs=2)
                    nc.vector.tensor_mul(zsq[:], zsl, zsl)
                    nc.tensor.matmul(
                        sum_ps[:], zsl, mct[:, m : m + 1],
                        start=(m == 0), stop=(m == MB - 1),
                    )
                    nc.tensor.matmul(
                        sq_ps[:], zsq[:], mct[:, m : m + 1],
                        start=(m == 0), stop=(m == MB - 1),
                    )
                st = pcst.tile([F, 4], f32, name="st", bufs=2)
                st_sb[l] = st
                nc.scalar.copy(st[:, 0:1], sum_ps[:])
                nc.scalar.copy(st[:, 1:2], sq_ps[:])
                nc.sync.dma_start(sti[l][:], st[:, 0:2])
                nc.gpsimd.collective_compute(
                    "AllReduce", Alu.add, replica_groups=rg,
                    ins=[sti[l][:]], outs=[sto[l][:]],
                )
                nc.sync.dma_start(st[:, 2:4], sto[l][:])
                return zt_next

            def st_to_sbuf(sts):
                stt = pst.tile([F, RPC], f32, name="stt", bufs=2)
                for ci, (off, cw) in enumerate(CHUNKS):
                    if ci % 2 == 0:
                        nc.scalar.copy(stt[:, off : off + cw], sts[ci][:])
                    else:
                        nc.vector.tensor_copy(stt[:, off : off + cw], sts[ci][:])
                return stt

            # ================= layer 1 =================
            zx = []
            for r in range(NC_):
                t = pz.tile([128, MB, F], bf16, name="zt", bufs=16)
                nc.sync.dma_start(t[:], xz_in[r])
                zx.append(t)
            sts = spmm(0, zx)
            stt = st_to_sbuf(sts)
            zown0 = pz.tile([128, MB, F], bf16, name="zown", bufs=2)
            linear_block(0, stt, zown0)
            zt1 = stats_and_comms(0, zown0)
            bn_fold(0)

            # ================= layer 2 =================
            sts = spmm(1, zt1)
            v_matmul(0)
            stt = st_to_sbuf(sts)
            zown1 = pz.tile([128, MB, F], bf16, name="zown", bufs=2)
            linear_block(1, stt, zown1)
            zt2 = stats_and_comms(1, zown1)
            bn_fold(1)

            # ================= layer 3 =================
            sts = spmm(2, zt2)
            v_matmul(1)
            stt = st_to_sbuf(sts)
            linear_block(2, stt, None)

    # Hardware instructions encode a limited number of sync waits (DMA ops
    # take only 2). The Tile scheduler can emit more; spill the overflow onto
    # nofuse NoOps inserted just before the overloaded instruction.
    for fn in nc.m.functions:
        for bb in fn.blocks:
            lst = bb.instructions
            new = []
            for ins in lst:
                si = getattr(ins, "sync_info", None)
                ow = list(si.on_wait) if si is not None else []
                LIMIT = 1
                if len(ow) > LIMIT:
                    over, keep = ow[:-LIMIT], ow[-LIMIT:]
                    for j in range(0, len(over), LIMIT):
                        new.append(
                            mybir.InstNoOp(
                                name=f"{ins.name}-w{j}",
                                engine=ins.engine,
                                bass_nofuse=True,
                                sync_info=mybir.SyncInfo(
                                    on_wait=over[j : j + LIMIT], on_update=[]
                                ),
                            )
                        )
                    ins.sync_info = bass_rust.SyncInfo(
                        on_wait=keep, on_update=list(si.on_update)
                    )
                new.append(ins)
            lst[:] = new

    return nc


def _prep_inputs(x, edge_row, edge_col, edge_val, W1, b1, gamma2, beta2,
                 W2, b2, gamma3, beta3, W3, b3):
    bf = ml_dtypes.bfloat16
    A = np.zeros((NP, NP), dtype=np.float32)
    np.add.at(A, (edge_row, edge_col), edge_val)
    Abf = A.astype(bf)
    rowsum = Abf.astype(np.float32).sum(axis=1)  # [NP]

    x_pad = np.zeros((NP, F), dtype=np.float32)
    x_pad[:N_NODES] = x
    xz = np.ascontiguousarray(
        x_pad.reshape(NC_, MB, 128, F).transpose(0, 2, 1, 3)
    ).astype(bf)

    gb = np.stack([gamma2, beta2, gamma3, beta3], axis=1).astype(np.float32)
    bs = np.zeros((3, F), dtype=np.float32)
    bs[0] = b1
    bs[1] = b2
    bs[2, :FO] = b3

    real = (np.arange(NP) < N_NODES).astype(np.float32)

    in_maps = []
    for c in range(NC_):
        rows = slice(c * RPC, (c + 1) * RPC)
        a_c = np.ascontiguousarray(Abf[rows, :].T).reshape(KB, 128, RPC)
        rsm = np.stack([rowsum[rows], real[rows]], axis=0).astype(np.float32)
        mc = np.ascontiguousarray(
            real[rows].reshape(MB, 128).T
        ).astype(bf)  # [128, MB]
        in_maps.append({
            "a": a_c,
            "xz": xz,
            "w1": np.asarray(W1, np.float32),
            "w2": np.asarray(W2, np.float32),
            "w3": np.asarray(W3, np.float32),
            "gb": gb,
            "bs": bs,
            "rsm": rsm,
            "mc": mc,
        })
    return in_maps


def kernel(x, edge_row, edge_col, edge_val, W1, b1, gamma2, beta2,
           W2, b2, gamma3, beta3, W3, b3):
    global LAST_RESULTS
    from concourse.bass_utils import run_bass_kernel_spmd

    x = np.asarray(x, np.float32)
    edge_row = np.asarray(edge_row).astype(np.int64)
    edge_col = np.asarray(edge_col).astype(np.int64)
    edge_val = np.asarray(edge_val, np.float32)

    if "nc" not in _cache:
        _cache["nc"] = _build()
    nc = _cache["nc"]

    in_maps = _prep_inputs(
        x, edge_row, edge_col, edge_val,
        np.asarray(W1, np.float32), np.asarray(b1, np.float32),
        np.asarray(gamma2, np.float32), np.asarray(beta2, np.float32),
        np.asarray(W2, np.float32), np.asarray(b2, np.float32),
        np.asarray(gamma3, np.float32), np.asarray(beta3, np.float32),
        np.asarray(W3, np.float32), np.asarray(b3, np.float32),
    )
    res = run_bass_kernel_spmd(nc, in_maps, core_ids=list(range(NC_)))
    LAST_RESULTS = res
    z3 = np.concatenate(
        [np.asarray(r["out"], dtype=np.float32) for r in res.results], axis=0
    )[:N_NODES]
    m = z3.max(axis=1, keepdims=True)
    e = z3 - m
    out = e - np.log(np.exp(e).sum(axis=1, keepdims=True))
    return out.astype(np.float32)


# revision 9
# speedup vs baseline: 1.1074x; 1.1074x over previous
import os
import sys

sys.path.insert(0, "/opt/trn_rl_repo")

import numpy as np
import ml_dtypes

N_NODES = 10000
NP = 10240          # padded node count (80 * 128)
F = 128
FO = 40
BN_EPS = 1e-5
NC_ = 8
RPC = NP // NC_     # 1280 rows per core
MB = RPC // 128     # 10 m-blocks per core
HB = MB // 2        # half (5 m-blocks) per AllGather half
KB = NP // 128      # 80 k-blocks
R_RES = 32          # A k-blocks resident in SBUF across layers
G = 8               # k-group size for chunk-pass interleaving
CHUNKS = [(0, 512), (512, 512), (1024, 256)]
ZC = HB * F         # 640 z columns per half
ZT = ZC + 4         # half-2 payload: z + 4 bf16 cols (= 2 fp32 BN stats)

LAST_RESULTS = None
_cache = {}


def _build():
    import concourse.bass as bass
    import concourse.mybir as mybir
    from concourse.tile import TileContext
    from concourse.vector_clock import ScopedClock
    import bass_rust

    f32 = mybir.dt.float32
    bf16 = mybir.dt.bfloat16
    Alu = mybir.AluOpType
    ActF = mybir.ActivationFunctionType

    class TC(TileContext):
        # The stock final drain puts one sync-wait per outstanding semaphore
        # lane on a single instruction; hardware allows fewer. Overflow is
        # handled by the global wait-splitting post-pass below.
        def _drain_and_barrier(self, tick_clock, wait_clock):
            first = self.nc.sync.nop(nofuse=True, hint="final_wait")
            wait_clock.add_sem_waits(
                first.ins, ScopedClock({None: tick_clock.global_clock})
            )
            si = first.ins.sync_info
            if si is not None and len(si.on_wait) > 4:
                waits = list(si.on_wait)
                first.ins.sync_info = bass_rust.SyncInfo(
                    on_wait=waits[:4], on_update=list(si.on_update)
                )
                for i in range(4, len(waits), 4):
                    extra = self.nc.sync.nop(nofuse=True, hint=f"final_wait_{i}")
                    extra.ins.sync_info = bass_rust.SyncInfo(
                        on_wait=waits[i : i + 4], on_update=[]
                    )
            self.nc.sync.drain()
            self.nc.all_engine_barrier()
            assert self.sems is not None
            popped = self.nc._tile_sem_poison_stack.pop()
            assert popped is self._sem_poison
            self.nc.clear_and_free_semaphores(list(self.sems.allocated().values()))
            self.nc.all_engine_barrier()

    nc = bass.Bass(num_devices=NC_)
    a_in = nc.declare_dram_parameter("a", [KB, 128, RPC], bf16, isOutput=False)
    xz_in = nc.declare_dram_parameter("xz", [NC_, 128, MB, F], bf16, isOutput=False)
    w1_in = nc.declare_dram_parameter("w1", [F, F], f32, isOutput=False)
    w2_in = nc.declare_dram_parameter("w2", [F, F], f32, isOutput=False)
    w3_in = nc.declare_dram_parameter("w3", [F, FO], f32, isOutput=False)
    gb_in = nc.declare_dram_parameter("gb", [F, 4], f32, isOutput=False)
    bs_in = nc.declare_dram_parameter("bs", [3, F], f32, isOutput=False)
    rsm_in = nc.declare_dram_parameter("rsm", [2, RPC], f32, isOutput=False)
    mc_in = nc.declare_dram_parameter("mc", [128, MB], bf16, isOutput=False)
    out_p = nc.declare_dram_parameter("out", [128, MB, FO], f32, isOutput=True)

    rg = [list(range(NC_))]
    zb1 = [nc.dram_tensor(f"zb1_{l}", [128, ZC], bf16, kind="Internal") for l in range(2)]
    zb2 = [nc.dram_tensor(f"zb2_{l}", [128, ZT], bf16, kind="Internal") for l in range(2)]
    zg1 = [
        nc.dram_tensor(f"zg1_{l}", [NC_, 128, ZC], bf16, kind="Internal", addr_space="Shared")
        for l in range(2)
    ]
    zg2 = [
        nc.dram_tensor(f"zg2_{l}", [NC_, 128, ZT], bf16, kind="Internal", addr_space="Shared")
        for l in range(2)
    ]

    # k traversal: all half-1 blocks (m<5 of every rank) first, then half-2;
    # within each half, groups of G with the three chunk passes inside so
    # consecutive matmuls always carry distinct weights (background LDW).
    k_order = [r * MB + m for r in range(NC_) for m in range(HB)] + [
        r * MB + m for r in range(NC_) for m in range(HB, MB)
    ]
    k_groups = [k_order[i : i + G] for i in range(0, len(k_order), G)]

    with TC(nc) as tc:
        with (
            tc.tile_pool(name="ares", bufs=1) as pares,
            tc.tile_pool(name="astr", bufs=1) as pastr,
            tc.tile_pool(name="pz", bufs=1) as pz,
            tc.tile_pool(name="pst", bufs=1) as pst,
            tc.tile_pool(name="psq", bufs=1) as psq,
            tc.tile_pool(name="pcst", bufs=1) as pcst,
            tc.tile_pool(name="pout", bufs=1) as pout,
            tc.tile_pool(name="psS", bufs=1, space="PSUM") as psS,
            tc.tile_pool(name="psZ", bufs=1, space="PSUM") as psZ,
            tc.tile_pool(name="psT", bufs=1, space="PSUM") as psT,
        ):
            # ---- constants
            w1t = pcst.tile([F, F], f32, bufs=1)
            nc.sync.dma_start(w1t[:], w1_in[:])
            w2t = pcst.tile([F, F], f32, bufs=1)
            nc.sync.dma_start(w2t[:], w2_in[:])
            w3t = pcst.tile([F, F], f32, bufs=1)
            nc.sync.dma_start(w3t[:, 0:FO], w3_in[:])
            gbt = pcst.tile([F, 4], f32, bufs=1)
            nc.sync.dma_start(gbt[:], gb_in[:])
            rsmt = pcst.tile([2, RPC], f32, bufs=1)
            nc.sync.dma_start(rsmt[:], rsm_in[:])
            mct = pcst.tile([128, MB], bf16, bufs=1)
            nc.sync.dma_start(mct[:], mc_in[:])
            rhs2 = []
            for l in range(3):
                t = pcst.tile([2, F], f32, name="rhs2t", bufs=3)
                if l == 0:
                    nc.vector.memset(t[0:1, :], 0.0)
                nc.sync.dma_start(t[1:2, :], bs_in[l : l + 1, :])
                rhs2.append(t)

            a_res = {}
            wp = [w1t, None, None]
            bn_sc = [None, None]

            def get_a_tile(l, k):
                if k < R_RES:
                    if l == 0:
                        at = pares.tile([128, RPC], bf16, name="ares_t", bufs=R_RES)
                        nc.sync.dma_start(at[:], a_in[k])
                        a_res[k] = at
                    return a_res[k]
                at = pastr.tile([128, RPC], bf16, name="astr_t", bufs=2 * G)
                nc.sync.dma_start(at[:], a_in[k])
                return at

            def spmm(l, zta, ztb):
                """S.T = (A_core @ Z).T in 3 PSUM chunks. zta/ztb: per-rank
                tiles holding m<5 / m>=5 z-blocks."""
                sts = [psS.tile([F, c], f32, name=f"stS{ci}", bufs=1)
                       for ci, (_, c) in enumerate(CHUNKS)]
                first = k_order[0]
                last = k_order[-1]
                for grp in k_groups:
                    atiles = {k: get_a_tile(l, k) for k in grp}
                    for ci, (off, cw) in enumerate(CHUNKS):
                        for k in grp:
                            r, m = k // MB, k % MB
                            if m < HB:
                                lhs = zta[r][:, m * F : (m + 1) * F]
                            else:
                                lhs = ztb[r][:, (m - HB) * F : (m - HB + 1) * F]
                            nc.tensor.matmul(
                                sts[ci][:],
                                lhs,
                                atiles[k][:, off : off + cw],
                                start=(k == first),
                                stop=(k == last),
                            )
                return sts

            def st_to_sbuf(sts):
                stt = pst.tile([F, RPC], f32, name="stt", bufs=2)
                for ci, (off, cw) in enumerate(CHUNKS):
                    if ci % 2 == 0:
                        nc.scalar.copy(stt[:, off : off + cw], sts[ci][:])
                    else:
                        nc.vector.tensor_copy(stt[:, off : off + cw], sts[ci][:])
                return stt

            def v_matmul(l):
                """v = shift.T @ W_next -> row 0 of rhs2[l+1]. Emitted after the
                next layer's SpMM so the PE never stalls waiting on BN stats."""
                _, shift = bn_sc[l]
                wsrc = w2t if l == 0 else w3t
                cols = F if l == 0 else FO
                vp = psZ.tile([1, F], f32, name="vp", bufs=1)
                nc.tensor.matmul(vp[:, 0:cols], shift, wsrc[:, 0:cols], start=True, stop=True)
                nc.scalar.copy(rhs2[l + 1][0:1, 0:cols], vp[:, 0:cols])

            def linear_rows(l, stt, lo, hi, dst):
                """zpre = S @ W_eff + rowsum x v + b for m in [lo,hi); relu into
                dst (bf16, z layout) or copy into dst (fp32 out tile, l==2)."""
                cols = F if l < 2 else FO
                for m in range(lo, hi):
                    zp = psZ.tile([128, cols], f32, name="zp", bufs=2)
                    nc.tensor.matmul(
                        zp[:], stt[:, m * 128 : (m + 1) * 128], wp[l][:, 0:cols],
                        start=True, stop=False,
                    )
                    nc.tensor.matmul(
                        zp[:], rsmt[:, m * 128 : (m + 1) * 128], rhs2[l][:, 0:cols],
                        start=False, stop=True,
                    )
                    if l < 2:
                        nc.scalar.activation(
                            dst[:, (m - lo) * F : (m - lo + 1) * F], zp[:], ActF.Relu
                        )
                    else:
                        nc.scalar.copy(dst[:, m, :], zp[:])

            def stats_into(zoa, zob):
                """BN partial sums over this core's real rows, written as raw
                fp32 bits into the tail bf16 columns of zob."""
                sum_ps = psT.tile([F, 1], f32, name="sum_ps", bufs=1)
                sq_ps = psT.tile([F, 1], f32, name="sq_ps", bufs=1)
                for m in range(MB):
                    zsl = (zoa[:, m * F : (m + 1) * F] if m < HB
                           else zob[:, (m - HB) * F : (m - HB + 1) * F])
                    zsq = psq.tile([128, F], bf16, name="zsq", bufs=2)
                    nc.vector.tensor_mul(zsq[:], zsl, zsl)
                    nc.tensor.matmul(sum_ps[:], zsl, mct[:, m : m + 1],
                                     start=(m == 0), stop=(m == MB - 1))
                    nc.tensor.matmul(sq_ps[:], zsq[:], mct[:, m : m + 1],
                                     start=(m == 0), stop=(m == MB - 1))
                z32 = zob.bitcast(f32)  # [128, ZT/2]
                nc.scalar.copy(z32[:, ZC // 2 : ZC // 2 + 1], sum_ps[:])
                nc.scalar.copy(z32[:, ZC // 2 + 1 : ZC // 2 + 2], sq_ps[:])

            def bn_fold(l, ztb_next):
                """Sum the 8 gathered BN partials locally, compute scale/shift,
                fold scale into next-layer weights. DVE/ACT only."""
                bn = pcst.tile([F, 12], f32, name="bn", bufs=2)
                acc = bn[:, 8:10]
                tmp = bn[:, 10:12]
                parts = [ztb_next[r].bitcast(f32)[:, ZC // 2 : ZC // 2 + 2]
                         for r in range(NC_)]
                nc.vector.tensor_add(acc, parts[0], parts[1])
                for r in range(2, NC_):
                    nc.vector.tensor_add(acc, acc, parts[r])
                mean, e2, msq, var = bn[:, 0:1], bn[:, 1:2], bn[:, 2:3], bn[:, 3:4]
                sd, rs, scale, shift = bn[:, 4:5], bn[:, 5:6], bn[:, 6:7], bn[:, 7:8]
                g_col = gbt[:, 2 * l : 2 * l + 1]
                b_col = gbt[:, 2 * l + 1 : 2 * l + 2]
                nc.vector.tensor_scalar_mul(mean, acc[:, 0:1], 1.0 / N_NODES)
                nc.vector.tensor_scalar_mul(e2, acc[:, 1:2], 1.0 / N_NODES)
                nc.vector.tensor_mul(msq, mean, mean)
                nc.vector.tensor_sub(var, e2, msq)
                nc.vector.tensor_scalar_add(var, var, BN_EPS)
                nc.scalar.sqrt(sd, var)
                nc.vector.reciprocal(rs, sd)
                nc.vector.tensor_mul(scale, g_col, rs)
                nc.vector.scalar_tensor_tensor(shift, mean, scale, b_col,
                                               Alu.mult, Alu.subtract)
                nc.vector.tensor_scalar_mul(shift, shift, -1.0)
                bn_sc[l] = (scale, shift)
                wsrc = w2t if l == 0 else w3t
                cols = F if l == 0 else FO
                w_eff = pcst.tile([F, F], f32, name="w_eff", bufs=2)
                nc.vector.tensor_scalar_mul(w_eff[:, 0:cols], wsrc[:, 0:cols], scale)
                wp[l + 1] = w_eff

            def gather_z(l):
                """DMA the AllGathered halves into per-rank SBUF tiles."""
                zta, ztb = [], []
                for r in range(NC_):
                    ta = pz.tile([128, ZC], bf16, name="zta", bufs=16)
                    nc.sync.dma_start(ta[:], zg1[l][r])
                    zta.append(ta)
                for r in range(NC_):
                    tb = pz.tile([128, ZT], bf16, name="ztb", bufs=16)
                    nc.sync.dma_start(tb[:], zg2[l][r])
                    ztb.append(tb)
                return zta, ztb

            # ================= layer 1 =================
            zta, ztb = [], []
            for r in range(NC_):
                ta = pz.tile([128, ZC], bf16, name="zta", bufs=16)
                nc.sync.dma_start(ta[:], xz_in[r, :, 0:HB, :])
                zta.append(ta)
            for r in range(NC_):
                tb = pz.tile([128, ZT], bf16, name="ztb", bufs=16)
                nc.sync.dma_start(tb[:, 0:ZC], xz_in[r, :, HB:MB, :])
                ztb.append(tb)

            for l in range(3):
                sts = spmm(l, zta, ztb)
                if l > 0:
                    v_matmul(l - 1)
                stt = st_to_sbuf(sts)
                if l < 2:
                    zoa = pz.tile([128, ZC], bf16, name="zoa", bufs=2)
                    zob = pz.tile([128, ZT], bf16, name="zob", bufs=2)
                    linear_rows(l, stt, 0, HB, zoa)
                    nc.sync.dma_start(zb1[l][:], zoa[:])
                    nc.gpsimd.collective_compute(
                        "AllGather", Alu.bypass, replica_groups=rg,
                        ins=[zb1[l][:]], outs=[zg1[l][:]],
                    )
                    linear_rows(l, stt, HB, MB, zob)
                    stats_into(zoa, zob)
                    nc.sync.dma_start(zb2[l][:], zob[:])
                    nc.gpsimd.collective_compute(
                        "AllGather", Alu.bypass, replica_groups=rg,
                        ins=[zb2[l][:]], outs=[zg2[l][:]],
                    )
                    zta, ztb = gather_z(l)
                    bn_fold(l, ztb)
                else:
                    ot = pout.tile([128, MB, FO], f32, name="ot", bufs=1)
                    linear_rows(l, stt, 0, MB, ot)
                    nc.sync.dma_start(out_p[:], ot[:])

    # Hardware instructions encode a very limited number of sync waits. The
    # Tile scheduler can emit more; spill the overflow onto nofuse NoOps
    # inserted just before the overloaded instruction.
    for fn in nc.m.functions:
        for bb in fn.blocks:
            lst = bb.instructions
            new = []
            for ins in lst:
                si = getattr(ins, "sync_info", None)
                ow = list(si.on_wait) if si is not None else []
                LIMIT = 1
                if len(ow) > LIMIT:
                    over, keep = ow[:-LIMIT], ow[-LIMIT:]
                    for j in range(0, len(over), LIMIT):
                        new.append(
                            mybir.InstNoOp(
                                name=f"{ins.name}-w{j}",
                                engine=ins.engine,
                                bass_nofuse=True,
                                sync_info=mybir.SyncInfo(
                                    on_wait=over[j : j + LIMIT], on_update=[]
                                ),
                            )
                        )
                    ins.sync_info = bass_rust.SyncInfo(
                        on_wait=keep, on_update=list(si.on_update)
                    )
                new.append(ins)
            lst[:] = new

    return nc


def _prep_inputs(x, edge_row, edge_col, edge_val, W1, b1, gamma2, beta2,
                 W2, b2, gamma3, beta3, W3, b3):
    bf = ml_dtypes.bfloat16
    A = np.zeros((NP, NP), dtype=np.float32)
    np.add.at(A, (edge_row, edge_col), edge_val)
    Abf = A.astype(bf)
    rowsum = Abf.astype(np.float32).sum(axis=1)  # [NP]

    x_pad = np.zeros((NP, F), dtype=np.float32)
    x_pad[:N_NODES] = x
    xz = np.ascontiguousarray(
        x_pad.reshape(NC_, MB, 128, F).transpose(0, 2, 1, 3)
    ).astype(bf)

    gb = np.stack([gamma2, beta2, gamma3, beta3], axis=1).astype(np.float32)
    bs = np.zeros((3, F), dtype=np.float32)
    bs[0] = b1
    bs[1] = b2
    bs[2, :FO] = b3

    real = (np.arange(NP) < N_NODES).astype(np.float32)

    in_maps = []
    for c in range(NC_):
        rows = slice(c * RPC, (c + 1) * RPC)
        a_c = np.ascontiguousarray(Abf[rows, :].T).reshape(KB, 128, RPC)
        rsm = np.stack([rowsum[rows], real[rows]], axis=0).astype(np.float32)
        mc = np.ascontiguousarray(
            real[rows].reshape(MB, 128).T
        ).astype(bf)  # [128, MB]
        in_maps.append({
            "a": a_c,
            "xz": xz,
            "w1": np.asarray(W1, np.float32),
            "w2": np.asarray(W2, np.float32),
            "w3": np.asarray(W3, np.float32),
            "gb": gb,
            "bs": bs,
            "rsm": rsm,
            "mc": mc,
        })
    return in_maps


def kernel(x, edge_row, edge_col, edge_val, W1, b1, gamma2, beta2,
           W2, b2, gamma3, beta3, W3, b3):
    global LAST_RESULTS
    from concourse.bass_utils import run_bass_kernel_spmd

    x = np.asarray(x, np.float32)
    edge_row = np.asarray(edge_row).astype(np.int64)
    edge_col = np.asarray(edge_col).astype(np.int64)
    edge_val = np.asarray(edge_val, np.float32)

    if "nc" not in _cache:
        _cache["nc"] = _build()
    nc = _cache["nc"]

    in_maps = _prep_inputs(
        x, edge_row, edge_col, edge_val,
        np.asarray(W1, np.float32), np.asarray(b1, np.float32),
        np.asarray(gamma2, np.float32), np.asarray(beta2, np.float32),
        np.asarray(W2, np.float32), np.asarray(b2, np.float32),
        np.asarray(gamma3, np.float32), np.asarray(beta3, np.float32),
        np.asarray(W3, np.float32), np.asarray(b3, np.float32),
    )
    res = run_bass_kernel_spmd(nc, in_maps, core_ids=list(range(NC_)))
    LAST_RESULTS = res
    z3 = np.concatenate(
        [
            np.asarray(r["out"], dtype=np.float32).transpose(1, 0, 2).reshape(RPC, FO)
            for r in res.results
        ],
        axis=0,
    )[:N_NODES]
    m = z3.max(axis=1, keepdims=True)
    e = z3 - m
    out = e - np.log(np.exp(e).sum(axis=1, keepdims=True))
    return out.astype(np.float32)


# revision 12
# speedup vs baseline: 1.3015x; 1.1752x over previous
import os
import sys

sys.path.insert(0, "/opt/trn_rl_repo")

import numpy as np
import ml_dtypes

N_NODES = 10000
NP = 10240          # padded node count (80 * 128)
F = 128
FO = 40
BN_EPS = 1e-5
NC_ = 8
RPC = NP // NC_     # 1280 rows per core
MB = RPC // 128     # 10 m-blocks per core
HB = MB // 2        # half (5 m-blocks) per AllGather half
KB = NP // 128      # 80 k-blocks
A_DT = os.environ.get("ADT", "e3m4")   # adjacency dtype: e3m4 | bf16
R_RES = KB if A_DT == "e3m4" else 32   # fp8 A fits fully resident in SBUF
G = 8
CHUNKS = [(0, 512), (512, 512), (1024, 256)]
ZC = HB * F         # 640 z columns per half
ZT = ZC + 4         # half-2 payload: z + 4 bf16 cols (= 2 fp32 BN stats)

LAST_RESULTS = None
_cache = {}


def _build():
    import concourse.bass as bass
    import concourse.mybir as mybir
    from concourse.tile import TileContext
    from concourse.vector_clock import ScopedClock
    import bass_rust

    f32 = mybir.dt.float32
    bf16 = mybir.dt.bfloat16
    Alu = mybir.AluOpType
    ActF = mybir.ActivationFunctionType

    class TC(TileContext):
        # The stock final drain puts one sync-wait per outstanding semaphore
        # lane on a single instruction; hardware allows fewer. Overflow is
        # handled by the global wait-splitting post-pass below.
        def _drain_and_barrier(self, tick_clock, wait_clock):
            first = self.nc.sync.nop(nofuse=True, hint="final_wait")
            wait_clock.add_sem_waits(
                first.ins, ScopedClock({None: tick_clock.global_clock})
            )
            si = first.ins.sync_info
            if si is not None and len(si.on_wait) > 4:
                waits = list(si.on_wait)
                first.ins.sync_info = bass_rust.SyncInfo(
                    on_wait=waits[:4], on_update=list(si.on_update)
                )
                for i in range(4, len(waits), 4):
                    extra = self.nc.sync.nop(nofuse=True, hint=f"final_wait_{i}")
                    extra.ins.sync_info = bass_rust.SyncInfo(
                        on_wait=waits[i : i + 4], on_update=[]
                    )
            self.nc.sync.drain()
            self.nc.all_engine_barrier()
            assert self.sems is not None
            popped = self.nc._tile_sem_poison_stack.pop()
            assert popped is self._sem_poison
            self.nc.clear_and_free_semaphores(list(self.sems.allocated().values()))
            self.nc.all_engine_barrier()

    nc = bass.Bass(num_devices=NC_)
    f8e3 = mybir.dt.float8e3
    a_dt = f8e3 if A_DT == "e3m4" else bf16
    a_in = nc.declare_dram_parameter("a", [KB, 128, RPC], a_dt, isOutput=False)
    xz_in = nc.declare_dram_parameter("xz", [NC_, 128, MB, F], bf16, isOutput=False)
    w1_in = nc.declare_dram_parameter("w1", [F, F], f32, isOutput=False)
    w2_in = nc.declare_dram_parameter("w2", [F, F], f32, isOutput=False)
    w3_in = nc.declare_dram_parameter("w3", [F, FO], f32, isOutput=False)
    gb_in = nc.declare_dram_parameter("gb", [F, 4], f32, isOutput=False)
    bs_in = nc.declare_dram_parameter("bs", [3, F], f32, isOutput=False)
    rsm_in = nc.declare_dram_parameter("rsm", [2, RPC], f32, isOutput=False)
    mc_in = nc.declare_dram_parameter("mc", [128, MB], bf16, isOutput=False)
    out_p = nc.declare_dram_parameter("out", [128, MB, FO], f32, isOutput=True)

    rg = [list(range(NC_))]
    zb1 = [nc.dram_tensor(f"zb1_{l}", [128, ZC], bf16, kind="Internal") for l in range(2)]
    zb2 = [nc.dram_tensor(f"zb2_{l}", [128, ZT], bf16, kind="Internal") for l in range(2)]
    zg1 = [
        nc.dram_tensor(f"zg1_{l}", [NC_, 128, ZC], bf16, kind="Internal", addr_space="Shared")
        for l in range(2)
    ]
    zg2 = [
        nc.dram_tensor(f"zg2_{l}", [NC_, 128, ZT], bf16, kind="Internal", addr_space="Shared")
        for l in range(2)
    ]

    # k traversal: all half-1 blocks (m<5 of every rank) first, then half-2;
    # within each half, groups of G with the three chunk passes inside so
    # consecutive matmuls always carry distinct weights (background LDW).
    k_order = [r * MB + m for r in range(NC_) for m in range(HB)] + [
        r * MB + m for r in range(NC_) for m in range(HB, MB)
    ]
    k_groups = [k_order[i : i + G] for i in range(0, len(k_order), G)]

    with TC(nc) as tc:
        with (
            tc.tile_pool(name="ares", bufs=1) as pares,
            tc.tile_pool(name="astr", bufs=1) as pastr,
            tc.tile_pool(name="pz", bufs=1) as pz,
            tc.tile_pool(name="pst", bufs=1) as pst,
            tc.tile_pool(name="psq", bufs=1) as psq,
            tc.tile_pool(name="pcst", bufs=1) as pcst,
            tc.tile_pool(name="pout", bufs=1) as pout,
            tc.tile_pool(name="psS", bufs=1, space="PSUM") as psS,
            tc.tile_pool(name="psZ", bufs=1, space="PSUM") as psZ,
            tc.tile_pool(name="psT", bufs=1, space="PSUM") as psT,
        ):
            # ---- constants
            salt = int(os.environ.get("KSALT", "0"))
            if salt:
                saltt = pcst.tile([1, 8 + (salt % 8)], f32, name="saltt", bufs=1)
                nc.vector.memset(saltt[:], float(salt))
            w1t = pcst.tile([F, F], f32, bufs=1)
            nc.sync.dma_start(w1t[:], w1_in[:])
            w2t = pcst.tile([F, F], f32, bufs=1)
            nc.sync.dma_start(w2t[:], w2_in[:])
            w3t = pcst.tile([F, F], f32, bufs=1)
            nc.sync.dma_start(w3t[:, 0:FO], w3_in[:])
            gbt = pcst.tile([F, 4], f32, bufs=1)
            nc.sync.dma_start(gbt[:], gb_in[:])
            rsmt = pcst.tile([2, RPC], f32, bufs=1)
            nc.sync.dma_start(rsmt[:], rsm_in[:])
            mct = pcst.tile([128, MB], bf16, bufs=1)
            nc.sync.dma_start(mct[:], mc_in[:])
            rhs2 = []
            for l in range(3):
                t = pcst.tile([2, F], f32, name="rhs2t", bufs=3)
                if l == 0:
                    nc.vector.memset(t[0:1, :], 0.0)
                nc.sync.dma_start(t[1:2, :], bs_in[l : l + 1, :])
                rhs2.append(t)

            a_res = {}
            wp = [w1t, None, None]
            bn_sc = [None, None]

            def get_a_tile(l, k):
                if k < R_RES:
                    if l == 0:
                        at = pares.tile([128, RPC], a_dt, name="ares_t", bufs=R_RES)
                        nc.sync.dma_start(at[:], a_in[k])
                        a_res[k] = at
                    return a_res[k]
                at = pastr.tile([128, RPC], a_dt, name="astr_t", bufs=2 * G)
                nc.sync.dma_start(at[:], a_in[k])
                return at

            def spmm(l, zta, ztb):
                """S.T = (A_core @ Z).T in 3 PSUM chunks. zta/ztb: per-rank
                tiles holding m<5 / m>=5 z-blocks."""
                sts = [psS.tile([F, c], f32, name=f"stS{ci}", bufs=1)
                       for ci, (_, c) in enumerate(CHUNKS)]
                first = k_order[0]
                last = k_order[-1]
                for k in k_order:
                    at = get_a_tile(l, k)
                    r, m = k // MB, k % MB
                    if m < HB:
                        lhs = zta[r][:, m * F : (m + 1) * F]
                    else:
                        lhs = ztb[r][:, (m - HB) * F : (m - HB + 1) * F]
                    for ci, (off, cw) in enumerate(CHUNKS):
                        nc.tensor.matmul(
                            sts[ci][:],
                            lhs,
                            at[:, off : off + cw],
                            start=(k == first),
                            stop=(k == last),
                        )
                return sts

            def st_to_sbuf(sts):
                stt = pst.tile([F, RPC], f32, name="stt", bufs=2)
                for ci, (off, cw) in enumerate(CHUNKS):
                    if ci % 2 == 0:
                        nc.scalar.copy(stt[:, off : off + cw], sts[ci][:])
                    else:
                        nc.vector.tensor_copy(stt[:, off : off + cw], sts[ci][:])
                return stt

            def v_matmul(l):
                """v = shift.T @ W_next -> row 0 of rhs2[l+1]. Emitted after the
                next layer's SpMM so the PE never stalls waiting on BN stats."""
                _, shift = bn_sc[l]
                wsrc = w2t if l == 0 else w3t
                cols = F if l == 0 else FO
                vp = psZ.tile([1, F], f32, name="vp", bufs=1)
                nc.tensor.matmul(vp[:, 0:cols], shift, wsrc[:, 0:cols], start=True, stop=True)
                nc.scalar.copy(rhs2[l + 1][0:1, 0:cols], vp[:, 0:cols])

            def linear_rows(l, stt, lo, hi, dst):
                """zpre = S @ W_eff + rowsum x v + b for m in [lo,hi); relu into
                dst (bf16, z layout) or copy into dst (fp32 out tile, l==2)."""
                cols = F if l < 2 else FO
                for m in range(lo, hi):
                    zp = psZ.tile([128, cols], f32, name="zp", bufs=2)
                    nc.tensor.matmul(
                        zp[:], stt[:, m * 128 : (m + 1) * 128], wp[l][:, 0:cols],
                        start=True, stop=False,
                    )
                    nc.tensor.matmul(
                        zp[:], rsmt[:, m * 128 : (m + 1) * 128], rhs2[l][:, 0:cols],
                        start=False, stop=True,
                    )
                    if l < 2:
                        nc.scalar.activation(
                            dst[:, (m - lo) * F : (m - lo + 1) * F], zp[:], ActF.Relu
                        )
                    else:
                        nc.scalar.copy(dst[:, m, :], zp[:])

            def stats_into(zoa, zob):
                """BN partial sums over this core's real rows, written as raw
                fp32 bits into the tail bf16 columns of zob."""
                sum_ps = psT.tile([F, 1], f32, name="sum_ps", bufs=1)
                sq_ps = psT.tile([F, 1], f32, name="sq_ps", bufs=1)
                for m in range(MB):
                    zsl = (zoa[:, m * F : (m + 1) * F] if m < HB
                           else zob[:, (m - HB) * F : (m - HB + 1) * F])
                    zsq = psq.tile([128, F], bf16, name="zsq", bufs=2)
                    nc.vector.tensor_mul(zsq[:], zsl, zsl)
                    nc.tensor.matmul(sum_ps[:], zsl, mct[:, m : m + 1],
                                     start=(m == 0), stop=(m == MB - 1))
                    nc.tensor.matmul(sq_ps[:], zsq[:], mct[:, m : m + 1],
                                     start=(m == 0), stop=(m == MB - 1))
                z32 = zob.bitcast(f32)  # [128, ZT/2]
                nc.scalar.copy(z32[:, ZC // 2 : ZC // 2 + 1], sum_ps[:])
                nc.scalar.copy(z32[:, ZC // 2 + 1 : ZC // 2 + 2], sq_ps[:])

            def bn_fold(l, ztb_next):
                """Sum the 8 gathered BN partials locally, compute scale/shift,
                fold scale into next-layer weights. DVE/ACT only."""
                bn = pcst.tile([F, 12], f32, name="bn", bufs=2)
                acc = bn[:, 8:10]
                tmp = bn[:, 10:12]
                parts = [ztb_next[r].bitcast(f32)[:, ZC // 2 : ZC // 2 + 2]
                         for r in range(NC_)]
                nc.vector.tensor_add(acc, parts[0], parts[1])
                for r in range(2, NC_):
                    nc.vector.tensor_add(acc, acc, parts[r])
                mean, e2, msq, var = bn[:, 0:1], bn[:, 1:2], bn[:, 2:3], bn[:, 3:4]
                sd, rs, scale, shift = bn[:, 4:5], bn[:, 5:6], bn[:, 6:7], bn[:, 7:8]
                g_col = gbt[:, 2 * l : 2 * l + 1]
                b_col = gbt[:, 2 * l + 1 : 2 * l + 2]
                nc.vector.tensor_scalar_mul(mean, acc[:, 0:1], 1.0 / N_NODES)
                nc.vector.tensor_scalar_mul(e2, acc[:, 1:2], 1.0 / N_NODES)
                nc.vector.tensor_mul(msq, mean, mean)
                nc.vector.tensor_sub(var, e2, msq)
                nc.vector.tensor_scalar_add(var, var, BN_EPS)
                nc.scalar.sqrt(sd, var)
                nc.vector.reciprocal(rs, sd)
                nc.vector.tensor_mul(scale, g_col, rs)
                nc.vector.scalar_tensor_tensor(shift, mean, scale, b_col,
                                               Alu.mult, Alu.subtract)
                nc.vector.tensor_scalar_mul(shift, shift, -1.0)
                bn_sc[l] = (scale, shift)
                wsrc = w2t if l == 0 else w3t
                cols = F if l == 0 else FO
                w_eff = pcst.tile([F, F], f32, name="w_eff", bufs=2)
                nc.vector.tensor_scalar_mul(w_eff[:, 0:cols], wsrc[:, 0:cols], scale)
                wp[l + 1] = w_eff

            def gather_z(l):
                """DMA the AllGathered halves into per-rank SBUF tiles."""
                zta, ztb = [], []
                for r in range(NC_):
                    ta = pz.tile([128, ZC], bf16, name="zta", bufs=16)
                    nc.sync.dma_start(ta[:], zg1[l][r])
                    zta.append(ta)
                for r in range(NC_):
                    tb = pz.tile([128, ZT], bf16, name="ztb", bufs=16)
                    nc.sync.dma_start(tb[:], zg2[l][r])
                    ztb.append(tb)
                return zta, ztb

            # ================= layer 1 =================
            zta, ztb = [], []
            for r in range(NC_):
                ta = pz.tile([128, ZC], bf16, name="zta", bufs=16)
                nc.sync.dma_start(ta[:], xz_in[r, :, 0:HB, :])
                zta.append(ta)
            for r in range(NC_):
                tb = pz.tile([128, ZT], bf16, name="ztb", bufs=16)
                nc.sync.dma_start(tb[:, 0:ZC], xz_in[r, :, HB:MB, :])
                ztb.append(tb)

            for l in range(3):
                sts = spmm(l, zta, ztb)
                if l > 0:
                    v_matmul(l - 1)
                stt = st_to_sbuf(sts)
                if l < 2:
                    zoa = pz.tile([128, ZC], bf16, name="zoa", bufs=2)
                    zob = pz.tile([128, ZT], bf16, name="zob", bufs=2)
                    linear_rows(l, stt, 0, HB, zoa)
                    nc.sync.dma_start(zb1[l][:], zoa[:])
                    nc.gpsimd.collective_compute(
                        "AllGather", Alu.bypass, replica_groups=rg,
                        ins=[zb1[l][:]], outs=[zg1[l][:]],
                    )
                    linear_rows(l, stt, HB, MB, zob)
                    stats_into(zoa, zob)
                    nc.sync.dma_start(zb2[l][:], zob[:])
                    nc.gpsimd.collective_compute(
                        "AllGather", Alu.bypass, replica_groups=rg,
                        ins=[zb2[l][:]], outs=[zg2[l][:]],
                    )
                    zta, ztb = gather_z(l)
                    bn_fold(l, ztb)
                else:
                    ot = pout.tile([128, MB, FO], f32, name="ot", bufs=1)
                    linear_rows(l, stt, 0, MB, ot)
                    nc.sync.dma_start(out_p[:], ot[:])

    # Consecutive LDWEIGHTS with identical weight operands reload the same
    # stationary tile; the PE keeps weights across matmuls, so convert the
    # redundant loads to NoOps (preserving their semaphore behavior).
    for fn in nc.m.functions:
        for bb in fn.blocks:
            lst = bb.instructions
            prev_sig = None
            for idx, ins in enumerate(lst):
                if ins.opcode != "Ldweights":
                    continue
                sig = str(ins.ins)
                if sig == prev_sig:
                    rep = mybir.InstNoOp(
                        name=ins.name,
                        engine=ins.engine,
                        bass_nofuse=True,
                        sync_info=ins.sync_info,
                    )
                    rep.merge_dependencies_from(ins)
                    lst[idx] = rep
                else:
                    prev_sig = sig

    # Hardware instructions encode a very limited number of sync waits. The
    # Tile scheduler can emit more; spill the overflow onto nofuse NoOps
    # inserted just before the overloaded instruction.
    for fn in nc.m.functions:
        for bb in fn.blocks:
            lst = bb.instructions
            new = []
            for ins in lst:
                si = getattr(ins, "sync_info", None)
                ow = list(si.on_wait) if si is not None else []
                LIMIT = 1
                if len(ow) > LIMIT:
                    over, keep = ow[:-LIMIT], ow[-LIMIT:]
                    for j in range(0, len(over), LIMIT):
                        new.append(
                            mybir.InstNoOp(
                                name=f"{ins.name}-w{j}",
                                engine=ins.engine,
                                bass_nofuse=True,
                                sync_info=mybir.SyncInfo(
                                    on_wait=over[j : j + LIMIT], on_update=[]
                                ),
                            )
                        )
                    ins.sync_info = bass_rust.SyncInfo(
                        on_wait=keep, on_update=list(si.on_update)
                    )
                new.append(ins)
            lst[:] = new

    return nc


def _prep_inputs(x, edge_row, edge_col, edge_val, W1, b1, gamma2, beta2,
                 W2, b2, gamma3, beta3, W3, b3):
    bf = ml_dtypes.bfloat16
    adt = ml_dtypes.float8_e3m4 if A_DT == "e3m4" else bf
    A = np.zeros((NP, NP), dtype=np.float32)
    np.add.at(A, (edge_row, edge_col), edge_val)
    Abf = A.astype(adt)
    rowsum = Abf.astype(np.float32).sum(axis=1)  # [NP]

    x_pad = np.zeros((NP, F), dtype=np.float32)
    x_pad[:N_NODES] = x
    xz = np.ascontiguousarray(
        x_pad.reshape(NC_, MB, 128, F).transpose(0, 2, 1, 3)
    ).astype(bf)

    gb = np.stack([gamma2, beta2, gamma3, beta3], axis=1).astype(np.float32)
    bs = np.zeros((3, F), dtype=np.float32)
    bs[0] = b1
    bs[1] = b2
    bs[2, :FO] = b3

    real = (np.arange(NP) < N_NODES).astype(np.float32)

    in_maps = []
    for c in range(NC_):
        rows = slice(c * RPC, (c + 1) * RPC)
        a_c = np.ascontiguousarray(Abf[rows, :].T).reshape(KB, 128, RPC)
        rsm = np.stack([rowsum[rows], real[rows]], axis=0).astype(np.float32)
        mc = np.ascontiguousarray(
            real[rows].reshape(MB, 128).T
        ).astype(bf)  # [128, MB]
        in_maps.append({
            "a": a_c,
            "xz": xz,
            "w1": np.asarray(W1, np.float32),
            "w2": np.asarray(W2, np.float32),
            "w3": np.asarray(W3, np.float32),
            "gb": gb,
            "bs": bs,
            "rsm": rsm,
            "mc": mc,
        })
    return in_maps


def _enable_ldw_opt():
    import concourse.bass_utils as bu

    if getattr(bu, "_ldw_opt_patched", False):
        return
    orig = bu.run_command

    def run_command(cmd, **kw):
        cmd = ["--enable-ldw-opt=true" if c == "--enable-ldw-opt=false" else c
               for c in cmd]
        return orig(cmd, **kw)

    bu.run_command = run_command
    bu._ldw_opt_patched = True


def kernel(x, edge_row, edge_col, edge_val, W1, b1, gamma2, beta2,
           W2, b2, gamma3, beta3, W3, b3):
    global LAST_RESULTS
    from concourse.bass_utils import run_bass_kernel_spmd

    if os.environ.get("LDWOPT") == "1":
        _enable_ldw_opt()

    x = np.asarray(x, np.float32)
    edge_row = np.asarray(edge_row).astype(np.int64)
    edge_col = np.asarray(edge_col).astype(np.int64)
    edge_val = np.asarray(edge_val, np.float32)

    if "nc" not in _cache:
        _cache["nc"] = _build()
    nc = _cache["nc"]

    in_maps = _prep_inputs(
        x, edge_row, edge_col, edge_val,
        np.asarray(W1, np.float32), np.asarray(b1, np.float32),
        np.asarray(gamma2, np.float32), np.asarray(beta2, np.float32),
        np.asarray(W2, np.float32), np.asarray(b2, np.float32),
        np.asarray(gamma3, np.float32), np.asarray(beta3, np.float32),
        np.asarray(W3, np.float32), np.asarray(b3, np.float32),
    )
    res = run_bass_kernel_spmd(nc, in_maps, core_ids=list(range(NC_)))
    LAST_RESULTS = res
    z3 = np.concatenate(
        [
            np.asarray(r["out"], dtype=np.float32).transpose(1, 0, 2).reshape(RPC, FO)
            for r in res.results
        ],
        axis=0,
    )[:N_NODES]
    m = z3.max(axis=1, keepdims=True)
    e = z3 - m
    out = e - np.log(np.exp(e).sum(axis=1, keepdims=True))
    return out.astype(np.float32)
